# revision 6
# baseline (speedup 1.0000x reference)
"""MoE routing gate kernel for Trainium2 (8 NeuronCores, data-parallel over tokens).

For x [16384, 2048] f32, weight [64, 2048] f32:
    logits = x @ weight.T; scores = softmax(logits)
    vals, idx = top_k(scores, 2); vals /= vals.sum(-1, keepdims=True)
    returns (vals f32 [16384,2], idx int32 [16384,2])

Per-core dataflow (2048 tokens, d=2048, E=64):
  Per token-tile t (128 tokens):
    - DMA x rows naturally [128tok, 2048d] (contiguous 1MB)
    - 16 PE transposes (128x128) -> PSUM, copied to SBUF xT tile
      [128d, 16k, 128tok] (copies split across DVE/ACT)
    - 16 accumulating matmuls: lhsT = xT[:,k,:] (stationary, fp32),
      rhs = wT[:,k,:] ([128d, 64e], streamed N=64) -> logits PSUM
      [128tok, 64e] directly in token-major layout.
    - softmax replicated exactly (exp with accumulated sum, reciprocal,
      scale) so underflow/tie semantics match the reference's top_k;
      top-2 via DVE max8/max_index on the rounded scores.
  MMs for tile t are emitted after transposes of tile t+1 so the PE
  always has queued work while the PSUM->SBUF copies complete.
weightT is derived on host (tiny) and passed as a separate input.
"""

import numpy as np

import concourse.bacc as bacc
import concourse.mybir as mybir
from concourse.bass_utils import run_bass_kernel_spmd
from concourse.masks import make_identity
from concourse.tile import TileContext

N_CORES = 8
T_FULL = 16384
D = 2048
E = 64
P = 128
T = T_FULL // N_CORES      # 2048 tokens per core
KC = D // P                # 16 contraction chunks
NT = T // P                # 16 token tiles per core

F32 = mybir.dt.float32
I32 = mybir.dt.int32
U32 = mybir.dt.uint32

_compiled = {}


def _build():
    nc = bacc.Bacc(
        "TRN2",
        target_bir_lowering=False,
        debug=False,
        enable_asserts=False,
        num_devices=N_CORES,
    )
    x = nc.declare_dram_parameter("x", [T, D], F32, isOutput=False)
    wt = nc.declare_dram_parameter("weightT", [D, E], F32, isOutput=False)
    out_v = nc.declare_dram_parameter("values", [T, 2], F32, isOutput=True)
    out_i = nc.declare_dram_parameter("indices", [T, 2], I32, isOutput=True)

    with TileContext(nc) as tc:
        with (
            tc.tile_pool(name="const", bufs=1) as const_pool,
            tc.tile_pool(name="xnat", bufs=8) as x_pool,
            tc.tile_pool(name="xt", bufs=3) as xt_pool,
            tc.tile_pool(name="small", bufs=6) as small_pool,
            tc.tile_pool(name="tpsum", bufs=6, space="PSUM") as tpsum_pool,
            tc.tile_pool(name="lpsum", bufs=2, space="PSUM") as lpsum_pool,
        ):
            ident = const_pool.tile([P, P], F32)
            make_identity(nc, ident)

            # wT [128d, KC, 64e]; DRAM [2048, 64] viewed as (k p) e -> p k e
            wT = const_pool.tile([P, KC, E], F32)
            nc.sync.dma_start(out=wT, in_=wt[:, :].rearrange("(k p) e -> p k e", p=P))

            def emit_transposes(t):
                """DMA x tile t and emit its 16 transposes + 4 copies."""
                xn = x_pool.tile([P, D], F32, name=f"xn{t}", tag="xn")
                nc.sync.dma_start(out=xn, in_=x[t * P:(t + 1) * P, :])
                xt = xt_pool.tile([P, KC, P], F32, name=f"xt{t}", tag="xt")
                for j in range(KC // 4):
                    pt = tpsum_pool.tile([P, 4, P], F32, name=f"pt{t}_{j}", tag="tp")
                    for c in range(4):
                        k = 4 * j + c
                        nc.tensor.transpose(
                            out=pt[:, c, :],
                            in_=xn[:, k * P:(k + 1) * P],
                            identity=ident,
                        )
                    dst = xt[:, 4 * j:4 * j + 4, :]
                    if j % 2 == 0:
                        nc.vector.tensor_copy(out=dst, in_=pt)
                    else:
                        nc.scalar.copy(out=dst, in_=pt)
                return xt

            def emit_mms_and_epilogue(t, xt):
                lp = lpsum_pool.tile([P, E], F32, name=f"lp{t}", tag="lp")
                for k in range(KC):
                    nc.tensor.matmul(
                        out=lp,
                        lhsT=xt[:, k, :],
                        rhs=wT[:, k, :],
                        start=(k == 0),
                        stop=(k == KC - 1),
                    )
                lt = small_pool.tile([P, E], F32, name=f"lt{t}", tag="lt")
                nc.vector.tensor_copy(out=lt, in_=lp)

                # softmax replicated: p = exp(l - l1) (+sum), s = p/Z
                m8l = small_pool.tile([P, 8], F32, name=f"m8l{t}", tag="m8l")
                sml = small_pool.tile([P, 4], F32, name=f"sml{t}", tag="sml")
                nc.vector.max(out=m8l, in_=lt)
                nc.vector.tensor_scalar_mul(sml[:, 0:1], m8l[:, 0:1], -1.0)
                nc.scalar.activation(
                    out=lt,
                    in_=lt,
                    func=mybir.ActivationFunctionType.Exp,
                    bias=sml[:, 0:1],
                    accum_out=sml[:, 1:2],
                )
                nc.vector.reciprocal(out=sml[:, 2:3], in_=sml[:, 1:2])
                nc.vector.tensor_scalar_mul(lt, lt, sml[:, 2:3])

                m8 = small_pool.tile([P, 8], F32, name=f"m8{t}", tag="m8")
                i8 = small_pool.tile([P, 8], U32, name=f"i8{t}", tag="i8")
                nc.vector.max(out=m8, in_=lt)
                nc.vector.max_index(out=i8, in_max=m8, in_values=lt)

                vals = small_pool.tile([P, 2], F32, name=f"vals{t}", tag="vals")
                idxs = small_pool.tile([P, 2], I32, name=f"idxs{t}", tag="idxs")
                nc.vector.tensor_add(out=sml[:, 3:4], in0=m8[:, 0:1], in1=m8[:, 1:2])
                nc.vector.reciprocal(out=sml[:, 3:4], in_=sml[:, 3:4])
                nc.vector.tensor_scalar_mul(vals, m8[:, 0:2], sml[:, 3:4])
                nc.vector.tensor_copy(out=idxs, in_=i8[:, 0:2])

                nc.sync.dma_start(out=out_v[t * P:(t + 1) * P, :], in_=vals)
                nc.sync.dma_start(out=out_i[t * P:(t + 1) * P, :], in_=idxs)

            # software pipeline: transposes of tile t+1 before MMs of tile t
            prev = None
            for t in range(NT):
                xt = emit_transposes(t)
                if prev is not None:
                    emit_mms_and_epilogue(t - 1, prev)
                prev = xt
            emit_mms_and_epilogue(NT - 1, prev)

    nc.compile()
    return nc


def _get_nc():
    if "nc" not in _compiled:
        _compiled["nc"] = _build()
    return _compiled["nc"]


def kernel(x: np.ndarray, weight: np.ndarray):
    x = np.ascontiguousarray(x, dtype=np.float32)
    weight = np.ascontiguousarray(weight, dtype=np.float32)
    wt = np.ascontiguousarray(weight.T)
    nc = _get_nc()
    in_maps = [
        {"x": x[i * T:(i + 1) * T], "weightT": wt} for i in range(N_CORES)
    ]
    res = run_bass_kernel_spmd(nc, in_maps, list(range(N_CORES)))
    values = np.concatenate([r["values"] for r in res.results], axis=0)
    indices = np.concatenate([r["indices"] for r in res.results], axis=0)
    return values, indices


# revision 11
# speedup vs baseline: 1.0904x; 1.0904x over previous
"""MoE routing gate kernel for Trainium2 (8 NeuronCores, data-parallel over tokens).

For x [16384, 2048] f32, weight [64, 2048] f32:
    logits = x @ weight.T; scores = softmax(logits)
    vals, idx = top_k(scores, 2); vals /= vals.sum(-1, keepdims=True)
    returns (vals f32 [16384,2], idx int32 [16384,2])

Per-core dataflow (2048 tokens, d=2048, E=64), slab = 512 tokens:
  - DMA x rows naturally [128tok, 2048d]; PE-transpose 128x128 blocks
    into PSUM; copy PSUM->SBUF (split DVE/ACT) building xT
    [128d, 16k, 512tok] per slab.
  - Matmuls: wT[k] [128d, 64e] stationary. The 64-expert output only
    fills half the PE array columns, so two matmuls run concurrently
    via tile_position column packing: token half A on col-strips 0-1
    -> PSUM[0:64], half B on strips 2-3 -> PSUM[64:128]. 16 k-chunks
    accumulate. This halves fp32 matmul stream time.
  - Transpose logits back to [128tok, 64e] tiles; replicate the
    reference softmax exactly (exp with accumulated sum, reciprocal,
    scale) so underflow/tie semantics match; top-2 via max8/max_index
    on the rounded scores; renorm; per-slab batched output DMAs.
"""

import numpy as np

import concourse.bacc as bacc
import concourse.mybir as mybir
from concourse.bass_utils import run_bass_kernel_spmd
from concourse.masks import make_identity
from concourse.tile import TileContext

N_CORES = 8
T_FULL = 16384
D = 2048
E = 64
P = 128
T = T_FULL // N_CORES      # 2048 tokens per core
KC = D // P                # 16 contraction chunks
SLAB = 512
NSLAB = T // SLAB          # 4
TPS = SLAB // P            # 4 token-tiles per slab
HALF = SLAB // 2           # 256 tokens per packed matmul

F32 = mybir.dt.float32
I32 = mybir.dt.int32
U32 = mybir.dt.uint32

_compiled = {}


def _build():
    nc = bacc.Bacc(
        "TRN2",
        target_bir_lowering=False,
        debug=False,
        enable_asserts=False,
        num_devices=N_CORES,
    )
    x = nc.declare_dram_parameter("x", [T, D], F32, isOutput=False)
    w = nc.declare_dram_parameter("weight", [E, D], F32, isOutput=False)
    out_v = nc.declare_dram_parameter("values", [T, 2], F32, isOutput=True)
    out_i = nc.declare_dram_parameter("indices", [T, 2], I32, isOutput=True)

    with TileContext(nc) as tc:
        with (
            tc.tile_pool(name="const", bufs=1) as const_pool,
            tc.tile_pool(name="xnat", bufs=8) as x_pool,
            tc.tile_pool(name="xt", bufs=2) as xt_pool,
            tc.tile_pool(name="small", bufs=4) as small_pool,
            tc.tile_pool(name="tpsum", bufs=4, space="PSUM") as tpsum_pool,
            tc.tile_pool(name="lpsum", bufs=4, space="PSUM") as lpsum_pool,
        ):
            ident = const_pool.tile([P, P], F32)
            make_identity(nc, ident)

            # wT [128d, KC, 64e] via on-chip transposes of natural weight
            w_sb = const_pool.tile([E, D], F32)
            nc.sync.dma_start(out=w_sb, in_=w[:, :])
            wT = const_pool.tile([P, KC, E], F32)
            for j in range(KC // 4):
                pt = tpsum_pool.tile([P, 4, E], F32, name=f"wpt{j}", tag="tp")
                for c in range(4):
                    k = 4 * j + c
                    nc.tensor.transpose(
                        out=pt[:, c, :],
                        in_=w_sb[:, k * P:(k + 1) * P],
                        identity=ident[:E, :E],
                    )
                nc.vector.tensor_copy(out=wT[:, 4 * j:4 * j + 4, :], in_=pt)

            copy_rr = 0
            for s in range(NSLAB):
                xt = xt_pool.tile([P, KC, SLAB], F32, name=f"xt{s}", tag="xt")
                for t in range(TPS):
                    row0 = (s * TPS + t) * P
                    xn = x_pool.tile([P, D], F32, name=f"xn{s}_{t}", tag="xn")
                    nc.sync.dma_start(out=xn, in_=x[row0:row0 + P, :])
                    for j in range(KC // 4):
                        pt = tpsum_pool.tile(
                            [P, 4, P], F32, name=f"pt{s}_{t}_{j}", tag="tp"
                        )
                        for c in range(4):
                            k = 4 * j + c
                            nc.tensor.transpose(
                                out=pt[:, c, :],
                                in_=xn[:, k * P:(k + 1) * P],
                                identity=ident,
                            )
                        dst = xt[:, 4 * j:4 * j + 4, t * P:(t + 1) * P]
                        if copy_rr % 16 in (1, 4, 7, 10, 13, 15):
                            nc.scalar.copy(out=dst, in_=pt)
                        else:
                            nc.vector.tensor_copy(out=dst, in_=pt)
                        copy_rr += 1

                # packed matmuls: token half A -> col strips 0-1 of the PE,
                # half B -> strips 2-3; separate PSUM banks so the two
                # accumulation groups don't share a zero region.
                lpA = lpsum_pool.tile([P, HALF], F32, name=f"lpA{s}", tag="lp")
                lpB = lpsum_pool.tile([P, HALF], F32, name=f"lpB{s}", tag="lp")
                for k in range(KC):
                    nc.tensor.matmul(
                        out=lpA[0:E, :],
                        lhsT=wT[:, k, :],
                        rhs=xt[:, k, 0:HALF],
                        start=(k == 0),
                        stop=(k == KC - 1),
                    )
                    nc.tensor.matmul(
                        out=lpB[0:E, :],
                        lhsT=wT[:, k, :],
                        rhs=xt[:, k, HALF:SLAB],
                        start=(k == 0),
                        stop=(k == KC - 1),
                    )
                lsb = small_pool.tile([P, HALF], F32, name=f"lsb{s}", tag="lsb")
                nc.vector.tensor_copy(out=lsb[0:E, :], in_=lpA[0:E, :])
                lsbB = small_pool.tile([E, HALF], F32, name=f"lsbB{s}", tag="lsbB")
                nc.vector.tensor_copy(out=lsbB, in_=lpB[0:E, :])

                # transpose logits back: 4x [64,128] -> [128,64]
                ltp = tpsum_pool.tile([P, TPS, E], F32, name=f"ltp{s}", tag="tp")
                for c in range(2):
                    nc.tensor.transpose(
                        out=ltp[:, c, :],
                        in_=lsb[0:E, c * P:(c + 1) * P],
                        identity=ident[:E, :E],
                    )
                    nc.tensor.transpose(
                        out=ltp[:, 2 + c, :],
                        in_=lsbB[:, c * P:(c + 1) * P],
                        identity=ident[:E, :E],
                    )
                lt = small_pool.tile([P, TPS, E], F32, name=f"lt{s}", tag="lt")
                nc.scalar.copy(out=lt, in_=ltp)

                # softmax replicated exactly; top-2 on rounded scores
                m8l = small_pool.tile([P, TPS, 8], F32, name=f"m8l{s}", tag="m8l")
                sml = small_pool.tile([P, TPS, 4], F32, name=f"sml{s}", tag="sml")
                m8 = small_pool.tile([P, TPS, 8], F32, name=f"m8{s}", tag="m8")
                i8 = small_pool.tile([P, TPS, 8], U32, name=f"i8{s}", tag="i8")
                for c in range(TPS):
                    nc.vector.max(out=m8l[:, c, :], in_=lt[:, c, :])
                nc.vector.tensor_scalar_mul(sml[:, :, 0], m8l[:, :, 0], -1.0)
                for c in range(TPS):
                    nc.scalar.activation(
                        out=lt[:, c, :],
                        in_=lt[:, c, :],
                        func=mybir.ActivationFunctionType.Exp,
                        bias=sml[:, c, 0:1],
                        accum_out=sml[:, c, 1:2],
                    )
                nc.vector.reciprocal(out=sml[:, :, 2], in_=sml[:, :, 1])
                for c in range(TPS):
                    nc.vector.tensor_scalar_mul(
                        lt[:, c, :], lt[:, c, :], sml[:, c, 2:3]
                    )
                    nc.vector.max(out=m8[:, c, :], in_=lt[:, c, :])
                    nc.vector.max_index(
                        out=i8[:, c, :], in_max=m8[:, c, :], in_values=lt[:, c, :]
                    )

                vals = small_pool.tile([P, TPS, 2], F32, name=f"vals{s}", tag="vals")
                idxs = small_pool.tile([P, TPS, 2], I32, name=f"idxs{s}", tag="idxs")
                nc.vector.tensor_add(out=sml[:, :, 3], in0=m8[:, :, 0], in1=m8[:, :, 1])
                nc.vector.reciprocal(out=sml[:, :, 3], in_=sml[:, :, 3])
                for c in range(TPS):
                    nc.vector.tensor_scalar_mul(
                        vals[:, c, :], m8[:, c, 0:2], sml[:, c, 3:4]
                    )
                nc.vector.tensor_copy(out=idxs, in_=i8[:, :, 0:2])

                for t in range(TPS):
                    row0 = (s * TPS + t) * P
                    nc.sync.dma_start(out=out_v[row0:row0 + P, :], in_=vals[:, t, :])
                    nc.sync.dma_start(out=out_i[row0:row0 + P, :], in_=idxs[:, t, :])

    nc.compile()
    return nc


def _get_nc():
    if "nc" not in _compiled:
        _compiled["nc"] = _build()
    return _compiled["nc"]


def kernel(x: np.ndarray, weight: np.ndarray):
    x = np.ascontiguousarray(x, dtype=np.float32)
    weight = np.ascontiguousarray(weight, dtype=np.float32)
    nc = _get_nc()
    in_maps = [
        {"x": x[i * T:(i + 1) * T], "weight": weight} for i in range(N_CORES)
    ]
    res = run_bass_kernel_spmd(nc, in_maps, list(range(N_CORES)))
    values = np.concatenate([r["values"] for r in res.results], axis=0)
    indices = np.concatenate([r["indices"] for r in res.results], axis=0)
    return values, indices
